# revision 49
# baseline (speedup 1.0000x reference)
"""Trainium2 Bass kernel for Tacotron2-style location-sensitive attention.

Reference computation (per batch row b):
    pq   = attention_hidden_state @ w_query.T                    (B, A)
    conv = Conv1d(attention_weights_cat, w_loc_conv, SAME)       (B, F, T)
    loc  = conv.T @ w_loc_dense.T                                (B, T, A)
    e    = tanh(pq[:,None,:] + loc + processed_memory) @ w_v     (B, T)
    w    = softmax(mask(e))                                      (B, T)
    ctx  = w @ memory                                            (B, E)

Sharding: data-parallel over batch B=64 across 8 NeuronCores (8 rows each).
Weights are tiny and replicated.

Device-side design:
  * conv+dense folded into one effective weight Weff[(c,k), a] (host-side
    weight-only transform); the conv becomes a K=62 matmul whose stationary
    operand is a [62, T] band matrix of shifted attention-weight rows,
    materialized per batch row with one SBUF->SBUF DMA from a staged copy.
  * big streams (attention weights, processed_memory, memory, Weff) travel
    in bf16; all accumulation is fp32 in PSUM, softmax + outputs are fp32.
  * energies are built in PSUM [t=128, 8 tiles x 128 a] (one 2-bank group):
    8 band matmuls (loc), 8 identity matmuls accumulating processed_memory,
    2 rank-1 matmuls broadcasting pq over the t partitions. One tanh (ACT),
    one multiply by v and one free-axis reduce (DVE) per group.
  * softmax skips the max-subtraction: |e| <= ||w_v||_1 ~= 2, so exp is
    safe. The mask is applied multiplicatively after exp (identical result).
  * context accumulates 16 [1,512] bf16 matmuls (lhsT = masked-exp column)
    and is scaled by 1/S at the end.
  * DMA issue is spread over the SP, Pool and ACT queues.
"""

import sys

if "/opt/trn_rl_repo" not in sys.path:
    sys.path.insert(0, "/opt/trn_rl_repo")

import ml_dtypes
import numpy as np

B, T = 64, 2048
RNN, EMB, ATT = 1024, 512, 128
NF, KS = 32, 31
NCORES = 8
BL = B // NCORES          # batch rows per core
P = 128
NT = T // P               # 16 t-tiles
GB = 8                    # t-tiles per energy PSUM group (2 banks)
NG = NT // GB             # energy groups per batch row
MB = 2                    # t-tiles per memory DMA
CK = 2 * KS               # 62 (c,k) pairs
PAD = (KS - 1) // 2       # 15
TPAD = T + 2 * PAD        # 2078
RC = RNN // P             # 8 chunks of the RNN dim

_CACHE = {}


def _build_program(stage=99):
    """stage: debug knob. 1=pq only, 3=energies, 4=+softmax, 99=full."""
    import concourse.bass as bass
    import concourse.mybir as mybir
    import concourse.tile as tile
    from concourse import bacc
    from concourse.masks import make_identity

    f32 = mybir.dt.float32
    bf16 = mybir.dt.bfloat16
    u8 = mybir.dt.uint8
    AF = mybir.ActivationFunctionType

    nc = bacc.Bacc("TRN2", target_bir_lowering=False, debug=False,
                   num_devices=NCORES)

    # aw_pad carries a third all-ones channel per batch row: the band matrix
    # then has rows 62..92 = ones; row 62 pairs with pq in the stationary
    # operand (folding the pq broadcast into the band matmul), rows 63..92
    # pair with zeros. pm arrives host-pre-tiled as (b, g, p, j, a) so each
    # group DMA has 2KB contiguous runs per partition.
    aw = nc.dram_tensor("aw_pad", [BL * 3, TPAD], bf16,
                        kind="ExternalInput").ap()
    pm = nc.dram_tensor("pm", [BL, NG, P, GB, ATT], bf16,
                        kind="ExternalInput").ap()
    mem = nc.dram_tensor("mem", [BL, T, EMB], bf16, kind="ExternalInput").ap()
    mask = nc.dram_tensor("mask", [BL, T], u8, kind="ExternalInput").ap()
    x = nc.dram_tensor("x", [BL, RNN], f32, kind="ExternalInput").ap()
    wqt = nc.dram_tensor("wqt", [RNN, ATT], f32, kind="ExternalInput").ap()
    weff = nc.dram_tensor("weff", [CK, ATT], bf16, kind="ExternalInput").ap()
    vv = nc.dram_tensor("v", [1, ATT], bf16, kind="ExternalInput").ap()
    ctxo = nc.dram_tensor("ctx", [BL, EMB], f32, kind="ExternalOutput").ap()
    wo = nc.dram_tensor("wout", [BL, T], f32, kind="ExternalOutput").ap()
    dbg = None
    if stage == 1:
        dbg = nc.dram_tensor("dbg", [BL, ATT], f32, kind="ExternalOutput").ap()
    elif stage == 3:
        dbg = nc.dram_tensor("dbg", [BL, P, NT], f32,
                             kind="ExternalOutput").ap()

    with tile.TileContext(nc) as tc:
        with (
            tc.tile_pool(name="consts", bufs=1) as consts,
            tc.tile_pool(name="xfull", bufs=2) as xfull_pool,
            tc.tile_pool(name="pmp", bufs=3) as pm_pool,
            tc.tile_pool(name="memp", bufs=40) as mem_pool,
            tc.tile_pool(name="tanhp", bufs=2) as tanh_pool,
            tc.tile_pool(name="junkp", bufs=2) as junk_pool,
            tc.tile_pool(name="energyp", bufs=2) as energy_pool,
            tc.tile_pool(name="smalls", bufs=3) as smalls,
            tc.tile_pool(name="pse", bufs=2, space="PSUM") as psum_e,
            tc.tile_pool(name="pss", bufs=2, space="PSUM") as psum_s,
            tc.tile_pool(name="psc", bufs=2, space="PSUM") as psum_c,
        ):
            # ---- constants (startup-critical loads first) ----
            x_sb = consts.tile([BL, RNN], f32)
            nc.sync.dma_start(out=x_sb, in_=x)
            # staged attention-weight rows (bf16), 3 channels per batch row
            aw_all = consts.tile([BL * 3, TPAD], bf16)
            nc.scalar.dma_start(out=aw_all, in_=aw)
            # wqt rearranged (r, a) -> [p, c, a] with r = c*128 + p
            wqt_sb = consts.tile([P, RC, ATT], f32)
            nc.sync.dma_start(
                out=wqt_sb,
                in_=bass.AP(tensor=wqt.tensor, offset=wqt.offset,
                            ap=[[ATT, P], [P * ATT, RC], [1, ATT]]))
            ident = consts.tile([P, P], f32)
            make_identity(nc, ident)
            ident_bf = consts.tile([P, P], bf16)
            nc.vector.tensor_copy(ident_bf, ident)
            ones_col = consts.tile([P, 1], f32)
            nc.vector.memset(ones_col, 1.0)
            ones_row = consts.tile([1, P], f32)
            nc.vector.memset(ones_row, 1.0)
            ones_row_bf = consts.tile([1, P], bf16)
            nc.vector.memset(ones_row_bf, 1.0)
            # v broadcast to 128 partitions x GB repeats
            v_sb8 = consts.tile([P, GB, ATT], bf16)
            nc.sync.dma_start(
                out=v_sb8,
                in_=bass.AP(tensor=vv.tensor, offset=vv.offset,
                            ap=[[0, P], [0, GB], [1, ATT]]))

            # ---- processed query: pq[b, a] = sum_r x[b, r] * wqt[r, a] ----
            xT_sb = consts.tile([P, RC, BL], f32)
            for c in range(RC):
                xT_ps = psum_s.tile([P, BL], f32, tag="pss", name=f"xT_ps{c}")
                nc.tensor.transpose(xT_ps, x_sb[:, c * P:(c + 1) * P],
                                    ident[:BL, :BL])
                nc.vector.tensor_copy(xT_sb[:, c, :], xT_ps)
            pq_ps = psum_s.tile([BL, ATT], f32, tag="pss", name="pq_ps")
            for c in range(RC):
                nc.tensor.matmul(pq_ps, lhsT=xT_sb[:, c, :],
                                 rhs=wqt_sb[:, c, :],
                                 start=(c == 0), stop=(c == RC - 1))
            pq_sb = consts.tile([BL, ATT], f32)
            nc.vector.tensor_copy(pq_sb, pq_ps)
            pq_bf = consts.tile([BL, ATT], bf16)
            nc.vector.tensor_copy(pq_bf, pq_ps)
            # stationary operand for the fused band matmul: Weff rows
            # replicated per batch row, pq[b] on row CK (pairs with the first
            # ones row of the band), zeros on rows CK+1.. (other ones rows)
            weffpq = consts.tile([3 * KS, BL, ATT], bf16)
            nc.vector.memset(weffpq, 0.0)
            nc.scalar.dma_start(
                out=weffpq[:CK],
                in_=bass.AP(tensor=weff.tensor, offset=weff.offset,
                            ap=[[ATT, CK], [0, BL], [1, ATT]]))
            nc.scalar.dma_start(
                out=weffpq[CK:CK + 1],
                in_=bass.AP(tensor=pq_bf.tensor, offset=pq_bf.offset,
                            ap=[[ATT, BL], [1, ATT]]))

            if stage == 1:
                nc.sync.dma_start(out=dbg, in_=pq_sb)

            # round-robin queues: big mem stream on the two HW rings' hosts,
            # smaller streams over three queues
            rr2_engines = [nc.sync, nc.gpsimd]
            rr3_engines = [nc.sync, nc.gpsimd, nc.scalar]
            rr_i = [0, 0]

            def rr2():
                e = rr2_engines[rr_i[0] % len(rr2_engines)]
                rr_i[0] += 1
                return e

            def rr3():
                e = rr3_engines[rr_i[1] % len(rr3_engines)]
                rr_i[1] += 1
                return e

            # energies/exp per batch chunk (separate tiles so a later
            # chunk's writes don't serialize an earlier chunk's softmax)
            NCHUNK = 4
            HB = BL // NCHUNK
            energy_h = [consts.tile([P, HB, NT], f32, name=f"energy_h{h}")
                        for h in range(NCHUNK)]
            exp_h = [consts.tile([P, HB, NT], f32, name=f"exp_h{h}")
                     for h in range(NCHUNK)]

            # ---- phase A: energies (keeps the ACT tanh table loaded) ----
            def phase_a(b):
                # band matrix of shifted attention-weight rows:
                # xfull[(c,k), t] = aw_pad[b, c, t + k]; split over 2 queues
                xfull = xfull_pool.tile([3 * KS, T], bf16, tag="xfull")
                for hh in range(2):
                    rr3().dma_start(
                        out=xfull[:, hh * (T // 2):(hh + 1) * (T // 2)],
                        in_=bass.AP(tensor=aw_all.tensor,
                                    offset=(aw_all.offset + b * 3 * TPAD
                                            + hh * (T // 2)),
                                    ap=[[TPAD, 3], [1, KS], [1, T // 2]]))

                for g in range(NG):
                    pm_sb = pm_pool.tile([P, GB, ATT], bf16, tag="pm")
                    rr3().dma_start(out=pm_sb, in_=pm[b, g])
                    t0 = g * GB * P
                    ps = psum_e.tile([P, GB * ATT], f32, tag="pse")
                    for j in range(GB):
                        nc.tensor.matmul(
                            ps[:, j * ATT:(j + 1) * ATT],
                            lhsT=xfull[:, t0 + j * P:t0 + (j + 1) * P],
                            rhs=weffpq[:, b, :], start=(j % 4 == 0),
                            stop=False, skip_group_check=True)
                    for j in range(GB):
                        nc.tensor.matmul(
                            ps[:, j * ATT:(j + 1) * ATT],
                            lhsT=ident_bf, rhs=pm_sb[:, j, :],
                            start=False, stop=(j == GB - 1),
                            skip_group_check=True)
                    tanh_sb = tanh_pool.tile([P, GB * ATT], bf16, tag="tanh")
                    nc.scalar.activation(tanh_sb, ps, AF.Tanh)
                    junk = junk_pool.tile([P, GB, ATT], bf16, tag="junk")
                    nc.vector.tensor_mul(
                        junk, tanh_sb.rearrange("p (g a) -> p g a", g=GB),
                        v_sb8)
                    nc.vector.reduce_sum(
                        energy_h[b // HB][:, b % HB, g * GB:(g + 1) * GB],
                        junk, axis=mybir.AxisListType.X)

            # batched weights output, one DMA at the end
            wout_all = consts.tile([NT, BL, P], f32)

            # ---- phase B: softmax + context per batch row ----
            def phase_b(b):
                # masked softmax (no max subtraction; |e| <= ~2)
                mask_sb = smalls.tile([NT, P], u8, tag="mask")
                rr2().dma_start(out=mask_sb,
                                in_=mask[b].rearrange("(q p) -> q p", p=P))
                notm = smalls.tile([NT, P], f32, tag="notm")
                nc.scalar.activation(notm, mask_sb, AF.Identity,
                                     bias=ones_col[:NT], scale=-1.0)
                notmT_ps = psum_s.tile([P, NT], f32, tag="pss", name="notmT")
                nc.tensor.transpose(notmT_ps, notm, ident[:NT, :NT])
                wexp = smalls.tile([P, NT], f32, tag="wexp")
                nc.vector.tensor_mul(wexp, exp_h[b // HB][:, b % HB, :],
                                     notmT_ps)
                wexp_bf = smalls.tile([P, NT], bf16, tag="wexpbf")
                nc.vector.tensor_copy(wexp_bf, wexp)
                colsum = smalls.tile([P, 1], f32, tag="colsum")
                nc.vector.reduce_sum(colsum, wexp, axis=mybir.AxisListType.X)
                s_ps = psum_s.tile([1, 1], f32, tag="pss", name="s_ps")
                nc.tensor.matmul(s_ps, lhsT=colsum, rhs=ones_col,
                                 start=True, stop=True)
                recip = smalls.tile([1, 1], f32, tag="recip")
                nc.vector.reciprocal(recip, s_ps)
                bc_ps = psum_s.tile([P, 1], f32, tag="pss", name="bc_ps")
                nc.tensor.matmul(bc_ps, lhsT=ones_row, rhs=recip,
                                 start=True, stop=True)
                bc_sb = smalls.tile([P, 1], f32, tag="bc")
                nc.scalar.copy(bc_sb, bc_ps)
                wnorm = smalls.tile([P, NT], f32, tag="wnorm")
                nc.vector.tensor_scalar_mul(wnorm, wexp, bc_sb)
                wnT_ps = psum_s.tile([NT, P], f32, tag="pss", name="wnT")
                nc.tensor.transpose(wnT_ps, wnorm, ident)
                nc.scalar.copy(wout_all[:, b, :], wnT_ps)

                # context: ctx[e] = sum_t wexp[t] * mem[t, e] / S
                ctx_ps = psum_c.tile([1, EMB], f32, tag="ctx")
                for tm in range(NT // MB):
                    mem_sb = mem_pool.tile([P, MB, EMB], bf16, tag="mem")
                    t0 = tm * MB * P
                    rr2().dma_start(
                        out=mem_sb,
                        in_=bass.AP(tensor=mem.tensor,
                                    offset=mem.offset + (b * T + t0) * EMB,
                                    ap=[[EMB, P], [P * EMB, MB], [1, EMB]]))
                    for j in range(MB):
                        ti = tm * MB + j
                        nc.tensor.matmul(ctx_ps,
                                         lhsT=wexp_bf[:, ti:ti + 1],
                                         rhs=mem_sb[:, j, :],
                                         start=(ti == 0), stop=(ti == NT - 1))
                ctx_sb = smalls.tile([1, EMB], f32, tag="ctxsb")
                nc.scalar.activation(ctx_sb, ctx_ps, AF.Copy, scale=recip)
                rr3().dma_start(out=ctxo[b:b + 1, :], in_=ctx_sb)

            # batch chunks: energies, one bulk exp, then softmax+context.
            # 2 ACT table loads per chunk instead of per batch row; a later
            # chunk's phase A overlaps an earlier chunk's phase B.
            if stage >= 3:
                for h in range(NCHUNK):
                    for b in range(h * HB, (h + 1) * HB):
                        phase_a(b)
                    nc.scalar.activation(exp_h[h], energy_h[h], AF.Exp)
                    if stage >= 4:
                        for b in range(h * HB, (h + 1) * HB):
                            phase_b(b)

            if stage == 3:
                for b in range(BL):
                    nc.sync.dma_start(out=dbg[b],
                                      in_=energy_h[b // HB][:, b % HB, :])

            if stage >= 4:
                nc.sync.dma_start(
                    out=bass.AP(tensor=wo.tensor, offset=wo.offset,
                                ap=[[P, NT], [T, BL], [1, P]]),
                    in_=wout_all)

    nc.compile()
    return nc


def get_program():
    if "nc" not in _CACHE:
        _CACHE["nc"] = _build_program()
    return _CACHE["nc"]


def make_in_maps(attention_hidden_state, memory, processed_memory,
                 attention_weights_cat, mask, w_query, w_loc_conv,
                 w_loc_dense, w_v):
    """Host-side prep: weight transforms + per-core batch shards."""
    bf = ml_dtypes.bfloat16
    ahs = np.ascontiguousarray(np.asarray(attention_hidden_state, np.float32))
    memory = np.asarray(memory, np.float32).astype(bf)
    # pre-tile processed_memory to (b, g, p, j, a) with t = g*GB*P + j*P + p
    # so each group's DMA reads 2KB contiguous runs per partition
    pm = (np.asarray(processed_memory, np.float32).astype(bf)
          .reshape(B, NG, GB, P, ATT).transpose(0, 1, 3, 2, 4))
    awc = np.asarray(attention_weights_cat, np.float32)
    mask_u8 = np.asarray(mask).astype(np.uint8)
    wq = np.asarray(w_query, np.float32)
    wc = np.asarray(w_loc_conv, np.float32)
    wd = np.asarray(w_loc_dense, np.float32)
    wv = np.asarray(w_v, np.float32)

    # Weff[(c,k), a] = sum_f wd[a, f] * wc[f, c, k], rows ordered c-major
    weff = np.einsum("af,fck->cka", wd, wc).reshape(CK, ATT).astype(bf)
    wqt = np.ascontiguousarray(wq.T)
    v_row = np.ascontiguousarray(wv.reshape(1, ATT)).astype(bf)
    aw_pad = np.ones((B, 3, TPAD), np.float32)
    aw_pad[:, :2, :] = 0.0
    aw_pad[:, :2, PAD:PAD + T] = awc
    aw_pad = aw_pad.astype(bf)

    in_maps = []
    for i in range(NCORES):
        sl = slice(i * BL, (i + 1) * BL)
        in_maps.append({
            "aw_pad": np.ascontiguousarray(
                aw_pad[sl].reshape(BL * 3, TPAD)),
            "pm": np.ascontiguousarray(pm[sl]),
            "mem": np.ascontiguousarray(memory[sl]),
            "mask": np.ascontiguousarray(mask_u8[sl]),
            "x": ahs[sl],
            "wqt": wqt,
            "weff": weff,
            "v": v_row,
        })
    return in_maps


def kernel(**inputs):
    from concourse.bass_utils import run_bass_kernel_spmd

    nc = get_program()
    in_maps = make_in_maps(**inputs)
    res = run_bass_kernel_spmd(nc, in_maps, core_ids=list(range(NCORES)))
    ctx = np.concatenate([res.results[i]["ctx"] for i in range(NCORES)], 0)
    w = np.concatenate([res.results[i]["wout"] for i in range(NCORES)], 0)
    return ctx, w


# revision 51
# speedup vs baseline: 1.5319x; 1.5319x over previous
"""Trainium2 Bass kernel for Tacotron2-style location-sensitive attention.

Reference computation (per batch row b):
    pq   = attention_hidden_state @ w_query.T                    (B, A)
    conv = Conv1d(attention_weights_cat, w_loc_conv, SAME)       (B, F, T)
    loc  = conv.T @ w_loc_dense.T                                (B, T, A)
    e    = tanh(pq[:,None,:] + loc + processed_memory) @ w_v     (B, T)
    w    = softmax(mask(e))                                      (B, T)
    ctx  = w @ memory                                            (B, E)

Sharding: data-parallel over batch B=64 across 8 NeuronCores (8 rows each).
Weights are tiny and replicated.

Device-side design:
  * conv+dense folded into one effective weight Weff[(c,k), a] (host-side
    weight-only transform); the conv becomes a K=62 matmul whose stationary
    operand is a [62, T] band matrix of shifted attention-weight rows,
    materialized per batch row with one SBUF->SBUF DMA from a staged copy.
  * big streams (attention weights, processed_memory, memory, Weff) travel
    in bf16; all accumulation is fp32 in PSUM, softmax + outputs are fp32.
  * energies are built in PSUM [t=128, 8 tiles x 128 a] (one 2-bank group):
    8 band matmuls (loc), 8 identity matmuls accumulating processed_memory,
    2 rank-1 matmuls broadcasting pq over the t partitions. One tanh (ACT),
    one multiply by v and one free-axis reduce (DVE) per group.
  * softmax skips the max-subtraction: |e| <= ||w_v||_1 ~= 2, so exp is
    safe. The mask is applied multiplicatively after exp (identical result).
  * context accumulates 16 [1,512] bf16 matmuls (lhsT = masked-exp column)
    and is scaled by 1/S at the end.
  * DMA issue is spread over the SP, Pool and ACT queues.
"""

import sys

if "/opt/trn_rl_repo" not in sys.path:
    sys.path.insert(0, "/opt/trn_rl_repo")

import ml_dtypes
import numpy as np

B, T = 64, 2048
RNN, EMB, ATT = 1024, 512, 128
NF, KS = 32, 31
NCORES = 8
BL = B // NCORES          # batch rows per core
P = 128
NT = T // P               # 16 t-tiles
GB = 8                    # t-tiles per energy PSUM group (2 banks)
NG = NT // GB             # energy groups per batch row
MB = 2                    # t-tiles per memory DMA
CK = 2 * KS               # 62 (c,k) pairs
PAD = (KS - 1) // 2       # 15
TPAD = T + 2 * PAD        # 2078
RC = RNN // P             # 8 chunks of the RNN dim

_CACHE = {}


def _build_program(stage=99):
    """stage: debug knob. 1=pq only, 3=energies, 4=+softmax, 99=full."""
    import concourse.bass as bass
    import concourse.mybir as mybir
    import concourse.tile as tile
    from concourse import bacc
    from concourse.masks import make_identity

    f32 = mybir.dt.float32
    bf16 = mybir.dt.bfloat16
    u8 = mybir.dt.uint8
    AF = mybir.ActivationFunctionType

    nc = bacc.Bacc("TRN2", target_bir_lowering=False, debug=False,
                   num_devices=NCORES)

    # aw_pad carries a third all-ones channel per batch row: the band matrix
    # then has rows 62..92 = ones; row 62 pairs with pq in the stationary
    # operand (folding the pq broadcast into the band matmul), rows 63..92
    # pair with zeros. pm arrives host-pre-tiled as (b, g, p, j, a) so each
    # group DMA has 2KB contiguous runs per partition.
    aw = nc.dram_tensor("aw_pad", [BL * 3, TPAD], bf16,
                        kind="ExternalInput").ap()
    pm = nc.dram_tensor("pm", [BL, NG, P, GB, ATT], bf16,
                        kind="ExternalInput").ap()
    mem = nc.dram_tensor("mem", [BL, T, EMB], bf16, kind="ExternalInput").ap()
    mask = nc.dram_tensor("mask", [BL, T], u8, kind="ExternalInput").ap()
    x = nc.dram_tensor("x", [BL, RNN], f32, kind="ExternalInput").ap()
    wqt = nc.dram_tensor("wqt", [RNN, ATT], f32, kind="ExternalInput").ap()
    weff = nc.dram_tensor("weff", [CK, ATT], bf16, kind="ExternalInput").ap()
    vv = nc.dram_tensor("v", [1, ATT], bf16, kind="ExternalInput").ap()
    ctxo = nc.dram_tensor("ctx", [BL, EMB], f32, kind="ExternalOutput").ap()
    wo = nc.dram_tensor("wout", [BL, T], f32, kind="ExternalOutput").ap()
    dbg = None
    if stage == 1:
        dbg = nc.dram_tensor("dbg", [BL, ATT], f32, kind="ExternalOutput").ap()
    elif stage == 3:
        dbg = nc.dram_tensor("dbg", [BL, P, NT], f32,
                             kind="ExternalOutput").ap()

    with tile.TileContext(nc) as tc:
        with (
            tc.tile_pool(name="consts", bufs=1) as consts,
            tc.tile_pool(name="xfull", bufs=2) as xfull_pool,
            tc.tile_pool(name="pmp", bufs=3) as pm_pool,
            tc.tile_pool(name="memp", bufs=40) as mem_pool,
            tc.tile_pool(name="tanhp", bufs=2) as tanh_pool,
            tc.tile_pool(name="junkp", bufs=2) as junk_pool,
            tc.tile_pool(name="energyp", bufs=2) as energy_pool,
            tc.tile_pool(name="smalls", bufs=3) as smalls,
            tc.tile_pool(name="pse", bufs=2, space="PSUM") as psum_e,
            tc.tile_pool(name="pss", bufs=2, space="PSUM") as psum_s,
            tc.tile_pool(name="psc", bufs=2, space="PSUM") as psum_c,
        ):
            # ---- constants (startup-critical loads first) ----
            x_sb = consts.tile([BL, RNN], f32)
            nc.sync.dma_start(out=x_sb, in_=x)
            # staged attention-weight rows (bf16), 3 channels per batch row
            aw_all = consts.tile([BL * 3, TPAD], bf16)
            nc.scalar.dma_start(out=aw_all, in_=aw)
            # wqt rearranged (r, a) -> [p, c, a] with r = c*128 + p
            wqt_sb = consts.tile([P, RC, ATT], f32)
            nc.gpsimd.dma_start(
                out=wqt_sb,
                in_=bass.AP(tensor=wqt.tensor, offset=wqt.offset,
                            ap=[[ATT, P], [P * ATT, RC], [1, ATT]]))
            ident = consts.tile([P, P], f32)
            make_identity(nc, ident)
            ident_bf = consts.tile([P, P], bf16)
            nc.vector.tensor_copy(ident_bf, ident)
            ones_col = consts.tile([P, 1], f32)
            nc.vector.memset(ones_col, 1.0)
            ones_row = consts.tile([1, P], f32)
            nc.vector.memset(ones_row, 1.0)
            ones_row_bf = consts.tile([1, P], bf16)
            nc.vector.memset(ones_row_bf, 1.0)
            # v broadcast to 128 partitions x GB repeats
            v_sb8 = consts.tile([P, GB, ATT], bf16)
            nc.sync.dma_start(
                out=v_sb8,
                in_=bass.AP(tensor=vv.tensor, offset=vv.offset,
                            ap=[[0, P], [0, GB], [1, ATT]]))

            # ---- processed query: pq[b, a] = sum_r x[b, r] * wqt[r, a] ----
            xT_sb = consts.tile([P, RC, BL], f32)
            for c in range(RC):
                xT_ps = psum_s.tile([P, BL], f32, tag="pss", name=f"xT_ps{c}")
                nc.tensor.transpose(xT_ps, x_sb[:, c * P:(c + 1) * P],
                                    ident[:BL, :BL])
                nc.vector.tensor_copy(xT_sb[:, c, :], xT_ps)
            pq_ps = psum_s.tile([BL, ATT], f32, tag="pss", name="pq_ps")
            for c in range(RC):
                nc.tensor.matmul(pq_ps, lhsT=xT_sb[:, c, :],
                                 rhs=wqt_sb[:, c, :],
                                 start=(c == 0), stop=(c == RC - 1))
            pq_sb = consts.tile([BL, ATT], f32)
            nc.vector.tensor_copy(pq_sb, pq_ps)
            pq_bf = consts.tile([BL, ATT], bf16)
            nc.vector.tensor_copy(pq_bf, pq_ps)
            # stationary operand for the fused band matmul: Weff rows
            # replicated per batch row, pq[b] on row CK (pairs with the first
            # ones row of the band), zeros on rows CK+1.. (other ones rows)
            weffpq = consts.tile([3 * KS, BL, ATT], bf16)
            nc.vector.memset(weffpq, 0.0)
            nc.scalar.dma_start(
                out=weffpq[:CK],
                in_=bass.AP(tensor=weff.tensor, offset=weff.offset,
                            ap=[[ATT, CK], [0, BL], [1, ATT]]))
            nc.scalar.dma_start(
                out=weffpq[CK:CK + 1],
                in_=bass.AP(tensor=pq_bf.tensor, offset=pq_bf.offset,
                            ap=[[ATT, BL], [1, ATT]]))

            if stage == 1:
                nc.sync.dma_start(out=dbg, in_=pq_sb)

            # round-robin queues: big mem stream on the two HW rings' hosts,
            # smaller streams over three queues
            rr2_engines = [nc.sync, nc.gpsimd]
            rr3_engines = [nc.sync, nc.gpsimd, nc.scalar]
            rr_i = [0, 0]

            def rr2():
                e = rr2_engines[rr_i[0] % len(rr2_engines)]
                rr_i[0] += 1
                return e

            def rr3():
                e = rr3_engines[rr_i[1] % len(rr3_engines)]
                rr_i[1] += 1
                return e

            # energies/exp per batch chunk (separate tiles so a later
            # chunk's writes don't serialize an earlier chunk's softmax)
            NCHUNK = 4
            HB = BL // NCHUNK
            energy_h = [consts.tile([P, HB, NT], f32, name=f"energy_h{h}")
                        for h in range(NCHUNK)]
            exp_h = [consts.tile([P, HB, NT], f32, name=f"exp_h{h}")
                     for h in range(NCHUNK)]

            # ---- phase A: energies (keeps the ACT tanh table loaded) ----
            def phase_a(b):
                # band matrix of shifted attention-weight rows:
                # xfull[(c,k), t] = aw_pad[b, c, t + k]; split over 2 queues
                xfull = xfull_pool.tile([3 * KS, T], bf16, tag="xfull")
                for hh in range(2):
                    rr3().dma_start(
                        out=xfull[:, hh * (T // 2):(hh + 1) * (T // 2)],
                        in_=bass.AP(tensor=aw_all.tensor,
                                    offset=(aw_all.offset + b * 3 * TPAD
                                            + hh * (T // 2)),
                                    ap=[[TPAD, 3], [1, KS], [1, T // 2]]))

                for g in range(NG):
                    pm_sb = pm_pool.tile([P, GB, ATT], bf16, tag="pm")
                    rr3().dma_start(out=pm_sb, in_=pm[b, g])
                    t0 = g * GB * P
                    ps = psum_e.tile([P, GB * ATT], f32, tag="pse")
                    for j in range(GB):
                        nc.tensor.matmul(
                            ps[:, j * ATT:(j + 1) * ATT],
                            lhsT=xfull[:, t0 + j * P:t0 + (j + 1) * P],
                            rhs=weffpq[:, b, :], start=(j % 4 == 0),
                            stop=False, skip_group_check=True)
                    for j in range(GB):
                        nc.tensor.matmul(
                            ps[:, j * ATT:(j + 1) * ATT],
                            lhsT=ident_bf, rhs=pm_sb[:, j, :],
                            start=False, stop=(j == GB - 1),
                            skip_group_check=True)
                    tanh_sb = tanh_pool.tile([P, GB * ATT], bf16, tag="tanh")
                    nc.scalar.activation(tanh_sb, ps, AF.Tanh)
                    junk = junk_pool.tile([P, GB, ATT], bf16, tag="junk")
                    nc.vector.tensor_mul(
                        junk, tanh_sb.rearrange("p (g a) -> p g a", g=GB),
                        v_sb8)
                    nc.vector.reduce_sum(
                        energy_h[b // HB][:, b % HB, g * GB:(g + 1) * GB],
                        junk, axis=mybir.AxisListType.X)

            # batched weights output, one DMA at the end
            wout_all = consts.tile([NT, BL, P], f32)

            # ---- phase B: softmax + context per batch row ----
            def phase_b(b):
                # masked softmax (no max subtraction; |e| <= ~2)
                mask_sb = smalls.tile([NT, P], u8, tag="mask")
                rr2().dma_start(out=mask_sb,
                                in_=mask[b].rearrange("(q p) -> q p", p=P))
                notm = smalls.tile([NT, P], f32, tag="notm")
                nc.scalar.activation(notm, mask_sb, AF.Identity,
                                     bias=ones_col[:NT], scale=-1.0)
                notmT_ps = psum_s.tile([P, NT], f32, tag="pss", name="notmT")
                nc.tensor.transpose(notmT_ps, notm, ident[:NT, :NT])
                wexp = smalls.tile([P, NT], f32, tag="wexp")
                nc.vector.tensor_mul(wexp, exp_h[b // HB][:, b % HB, :],
                                     notmT_ps)
                wexp_bf = smalls.tile([P, NT], bf16, tag="wexpbf")
                nc.vector.tensor_copy(wexp_bf, wexp)
                colsum = smalls.tile([P, 1], f32, tag="colsum")
                nc.vector.reduce_sum(colsum, wexp, axis=mybir.AxisListType.X)
                s_ps = psum_s.tile([1, 1], f32, tag="pss", name="s_ps")
                nc.tensor.matmul(s_ps, lhsT=colsum, rhs=ones_col,
                                 start=True, stop=True)
                recip = smalls.tile([1, 1], f32, tag="recip")
                nc.vector.reciprocal(recip, s_ps)
                bc_ps = psum_s.tile([P, 1], f32, tag="pss", name="bc_ps")
                nc.tensor.matmul(bc_ps, lhsT=ones_row, rhs=recip,
                                 start=True, stop=True)
                bc_sb = smalls.tile([P, 1], f32, tag="bc")
                nc.scalar.copy(bc_sb, bc_ps)
                wnorm = smalls.tile([P, NT], f32, tag="wnorm")
                nc.vector.tensor_scalar_mul(wnorm, wexp, bc_sb)
                wnT_ps = psum_s.tile([NT, P], f32, tag="pss", name="wnT")
                nc.tensor.transpose(wnT_ps, wnorm, ident)
                nc.scalar.copy(wout_all[:, b, :], wnT_ps)

                # context: ctx[e] = sum_t wexp[t] * mem[t, e] / S
                ctx_ps = psum_c.tile([1, EMB], f32, tag="ctx")
                for tm in range(NT // MB):
                    mem_sb = mem_pool.tile([P, MB, EMB], bf16, tag="mem")
                    t0 = tm * MB * P
                    rr2().dma_start(
                        out=mem_sb,
                        in_=bass.AP(tensor=mem.tensor,
                                    offset=mem.offset + (b * T + t0) * EMB,
                                    ap=[[EMB, P], [P * EMB, MB], [1, EMB]]))
                    for j in range(MB):
                        ti = tm * MB + j
                        nc.tensor.matmul(ctx_ps,
                                         lhsT=wexp_bf[:, ti:ti + 1],
                                         rhs=mem_sb[:, j, :],
                                         start=(ti == 0), stop=(ti == NT - 1))
                ctx_sb = smalls.tile([1, EMB], f32, tag="ctxsb")
                nc.scalar.activation(ctx_sb, ctx_ps, AF.Copy, scale=recip)
                rr3().dma_start(out=ctxo[b:b + 1, :], in_=ctx_sb)

            # batch chunks: energies, one bulk exp, then softmax+context.
            # 2 ACT table loads per chunk instead of per batch row; a later
            # chunk's phase A overlaps an earlier chunk's phase B.
            if stage >= 3:
                for h in range(NCHUNK):
                    for b in range(h * HB, (h + 1) * HB):
                        phase_a(b)
                    nc.scalar.activation(exp_h[h], energy_h[h], AF.Exp)
                    if stage >= 4:
                        for b in range(h * HB, (h + 1) * HB):
                            phase_b(b)

            if stage == 3:
                for b in range(BL):
                    nc.sync.dma_start(out=dbg[b],
                                      in_=energy_h[b // HB][:, b % HB, :])

            if stage >= 4:
                nc.gpsimd.dma_start(
                    out=bass.AP(tensor=wo.tensor, offset=wo.offset,
                                ap=[[P, NT], [T, BL], [1, P]]),
                    in_=wout_all)

    nc.compile()
    return nc


def get_program():
    if "nc" not in _CACHE:
        _CACHE["nc"] = _build_program()
    return _CACHE["nc"]


def make_in_maps(attention_hidden_state, memory, processed_memory,
                 attention_weights_cat, mask, w_query, w_loc_conv,
                 w_loc_dense, w_v):
    """Host-side prep: weight transforms + per-core batch shards."""
    bf = ml_dtypes.bfloat16
    ahs = np.ascontiguousarray(np.asarray(attention_hidden_state, np.float32))
    memory = np.asarray(memory, np.float32).astype(bf)
    # pre-tile processed_memory to (b, g, p, j, a) with t = g*GB*P + j*P + p
    # so each group's DMA reads 2KB contiguous runs per partition
    pm = (np.asarray(processed_memory, np.float32).astype(bf)
          .reshape(B, NG, GB, P, ATT).transpose(0, 1, 3, 2, 4))
    awc = np.asarray(attention_weights_cat, np.float32)
    mask_u8 = np.asarray(mask).astype(np.uint8)
    wq = np.asarray(w_query, np.float32)
    wc = np.asarray(w_loc_conv, np.float32)
    wd = np.asarray(w_loc_dense, np.float32)
    wv = np.asarray(w_v, np.float32)

    # Weff[(c,k), a] = sum_f wd[a, f] * wc[f, c, k], rows ordered c-major
    weff = np.einsum("af,fck->cka", wd, wc).reshape(CK, ATT).astype(bf)
    wqt = np.ascontiguousarray(wq.T)
    v_row = np.ascontiguousarray(wv.reshape(1, ATT)).astype(bf)
    aw_pad = np.ones((B, 3, TPAD), np.float32)
    aw_pad[:, :2, :] = 0.0
    aw_pad[:, :2, PAD:PAD + T] = awc
    aw_pad = aw_pad.astype(bf)

    in_maps = []
    for i in range(NCORES):
        sl = slice(i * BL, (i + 1) * BL)
        in_maps.append({
            "aw_pad": np.ascontiguousarray(
                aw_pad[sl].reshape(BL * 3, TPAD)),
            "pm": np.ascontiguousarray(pm[sl]),
            "mem": np.ascontiguousarray(memory[sl]),
            "mask": np.ascontiguousarray(mask_u8[sl]),
            "x": ahs[sl],
            "wqt": wqt,
            "weff": weff,
            "v": v_row,
        })
    return in_maps


def kernel(**inputs):
    from concourse.bass_utils import run_bass_kernel_spmd

    nc = get_program()
    in_maps = make_in_maps(**inputs)
    res = run_bass_kernel_spmd(nc, in_maps, core_ids=list(range(NCORES)))
    ctx = np.concatenate([res.results[i]["ctx"] for i in range(NCORES)], 0)
    w = np.concatenate([res.results[i]["wout"] for i in range(NCORES)], 0)
    return ctx, w
